# revision 4
# baseline (speedup 1.0000x reference)
"""ContrastiveKernelLoss on 8 Trainium2 cores.

Math: for each layer, D[i,j] = ||I - inv(kn_i) @ kn_j||_F over the n x n pair
grid.  ||I - A B||_F^2 = d - 2*tr(A B) + ||A B||_F^2 expands into dot products
over 2*d^2+1 per-kernel features, so the whole grid is one skinny matmul:
  X = W^T R,  W rows = [-2*vec(inv_i); vec(inv_i^T inv_i); d],
              R rows = [vec(kn_j^T);  vec(kn_j kn_j^T);   1]
The loss only needs the strict lower triangle (combined = tril(D) + triu(D^T)),
so each core computes a balanced subset of lower-triangle 128x512 tiles
("slots").  Upper/diagonal positions are pushed out of the hinge by adding
BIG via a third matmul (identity lhsT @ mask), masks generated on-device from
iota >= per-row threshold.  Per tile: PE matmul -> ACT sqrt -> DVE min(s,1)
with fused row-sum.  sum(relu(1-D)) = n_cols - sum(min(s,1)) exactly, because
inactive entries contribute exact 1.0.  Host reduces in float64.
"""

import numpy as np
from contextlib import ExitStack

EPS = 1e-8
# Masked (upper/diagonal) grid entries get +BIG before sqrt.  Must dominate
# the f32 matmul rounding error of the diagonal cancellation, which can reach
# ~1e5 for near-singular kernels (feature products up to ~2e11).
BIG = 1e8
N0, D0 = 2048, 3
N1, D1 = 1024, 5
F0 = 2 * D0 * D0 + 1   # 19
F1 = 2 * D1 * D1 + 1   # 51
NSLOT = 7              # 5 layer0 slots + 2 layer1 slots per core
N_CORES = 8

TRACE = False          # test harness sets True to capture an NTFF profile
LAST_RESULT = None     # BassKernelResults of the most recent run

_cached_nc = None


def _features(kernels, d):
    """Per-kernel features (float64 prep, float32 out): W (F, n), R (F, n)."""
    k = np.asarray(kernels, np.float64)
    n = k.shape[0]
    norms = np.sqrt((k * k).sum(axis=(1, 2), keepdims=True)) + EPS
    kn = k / norms
    inv = np.linalg.inv(kn)
    G = np.einsum('iba,ibc->iac', inv, inv)        # inv^T @ inv
    H = np.einsum('iab,icb->iac', kn, kn)          # kn @ kn^T
    W = np.concatenate([
        -2.0 * inv.reshape(n, -1),
        G.reshape(n, -1),
        np.full((n, 1), float(d)),
    ], axis=1).T
    R = np.concatenate([
        np.transpose(kn, (0, 2, 1)).reshape(n, -1),
        H.reshape(n, -1),
        np.ones((n, 1)),
    ], axis=1).T
    return (np.ascontiguousarray(W, np.float32),
            np.ascontiguousarray(R, np.float32))


def _core_plan(core):
    """Slot list for one core: (layer, row_tile, col_chunk, thr_or_None).

    Layer0 (16 row tiles of 128, chunks of 512): core pairs (k,12+k)/(k,k+4)
    give every core 2 diagonal slots + 3 full slots.  Layer1: tile k diag slot
    + (full chunk | fully-masked dummy).  thr is the per-partition column
    threshold: columns n >= thr are excluded (+BIG).
    """
    tA, tB = (core, 12 + core) if core < 4 else (core, core + 4)
    slots = []
    for t in (tA, tB):                       # slots 0,1: layer0 diagonal tiles
        slots.append(("l0", t, t // 4, 128.0 * (t % 4) + np.arange(128)))
    fulls = [(tA, c) for c in range(tA // 4)] + [(tB, c) for c in range(tB // 4)]
    assert len(fulls) == 3
    for (t, c) in fulls:                     # slots 2-4: layer0 full chunks
        slots.append(("l0", t, c, None))
    t = core                                 # slot 5: layer1 diagonal tile
    slots.append(("l1", t, t // 4, 128.0 * (t % 4) + np.arange(128)))
    if core >= 4:                            # slot 6: real full chunk
        slots.append(("l1", t, 0, np.full(128, 600.0)))
    else:                                    # slot 6: fully-masked dummy
        slots.append(("l1", t, 0, np.zeros(128)))
    return slots


def _stage_core(core, W0, R0, W1, R1):
    """Build the per-core input map for the static SPMD program."""
    slots = _core_plan(core)
    w0 = np.empty((F0, 5 * 128), np.float32)
    r0 = np.empty((F0, 5 * 512), np.float32)
    thr = np.empty((128, 4), np.float32)
    mi = 0
    for s, (lay, t, c, th) in enumerate(slots[:5]):
        w0[:, 128 * s:128 * (s + 1)] = W0[:, 128 * t:128 * (t + 1)]
        r0[:, 512 * s:512 * (s + 1)] = R0[:, 512 * c:512 * (c + 1)]
        if th is not None:
            thr[:, mi] = th
            mi += 1
    assert mi == 2
    _, t5, c5, th5 = slots[5]
    _, t6, c6, th6 = slots[6]
    assert t6 == t5
    w1 = np.ascontiguousarray(W1[:, 128 * t5:128 * (t5 + 1)])
    r1 = np.empty((F1, 1024), np.float32)
    r1[:, 0:512] = R1[:, 512 * c5:512 * (c5 + 1)]
    r1[:, 512:1024] = R1[:, 512 * c6:512 * (c6 + 1)]
    thr[:, 2] = th5
    thr[:, 3] = th6
    return {
        "w0": w0, "r0": r0, "w1": w1, "r1": r1,
        "thr": thr, "ident": np.eye(128, dtype=np.float32),
    }


def _build_program():
    import concourse.bacc as bacc
    import concourse.tile as tile
    import concourse.mybir as mybir

    f32 = mybir.dt.float32
    nc = bacc.Bacc("TRN2")
    w0 = nc.declare_dram_parameter("w0", [F0, 5 * 128], f32, isOutput=False)
    r0 = nc.declare_dram_parameter("r0", [F0, 5 * 512], f32, isOutput=False)
    w1 = nc.declare_dram_parameter("w1", [F1, 128], f32, isOutput=False)
    r1 = nc.declare_dram_parameter("r1", [F1, 1024], f32, isOutput=False)
    thr = nc.declare_dram_parameter("thr", [128, 4], f32, isOutput=False)
    ident = nc.declare_dram_parameter("ident", [128, 128], f32, isOutput=False)
    acc = nc.declare_dram_parameter("acc", [128, NSLOT], f32, isOutput=True)

    with tile.TileContext(nc) as tc, ExitStack() as ctx:
        cpool = ctx.enter_context(tc.tile_pool(name="const", bufs=1))
        ppool = ctx.enter_context(
            tc.tile_pool(name="psum", bufs=4, space="PSUM"))
        spool = ctx.enter_context(tc.tile_pool(name="work", bufs=3))

        w0_t = cpool.tile([F0, 5 * 128], f32)
        nc.sync.dma_start(w0_t[:], w0.ap())
        r0_t = cpool.tile([F0, 5 * 512], f32)
        nc.sync.dma_start(r0_t[:], r0.ap())
        w1_t = cpool.tile([F1, 128], f32)
        nc.sync.dma_start(w1_t[:], w1.ap())
        r1_t = cpool.tile([F1, 1024], f32)
        nc.sync.dma_start(r1_t[:], r1.ap())
        thr_t = cpool.tile([128, 4], f32)
        nc.sync.dma_start(thr_t[:], thr.ap())
        id_t = cpool.tile([128, 128], f32)
        nc.sync.dma_start(id_t[:], ident.ap())

        iota_t = cpool.tile([128, 512], f32)
        nc.gpsimd.iota(iota_t[:], pattern=[[1, 512]], base=0,
                       channel_multiplier=0,
                       allow_small_or_imprecise_dtypes=True)
        masks = []
        for mi in range(4):
            m = cpool.tile([128, 512], f32, tag=f"mask{mi}")
            nc.vector.tensor_scalar(
                out=m[:], in0=iota_t[:], scalar1=thr_t[:, mi:mi + 1],
                scalar2=BIG, op0=mybir.AluOpType.is_ge,
                op1=mybir.AluOpType.mult)
            masks.append(m)

        acc_t = cpool.tile([128, NSLOT], f32)
        slot_defs = [
            (w0_t[:, 0:128],   r0_t[:, 0:512],     masks[0]),
            (w0_t[:, 128:256], r0_t[:, 512:1024],  masks[1]),
            (w0_t[:, 256:384], r0_t[:, 1024:1536], None),
            (w0_t[:, 384:512], r0_t[:, 1536:2048], None),
            (w0_t[:, 512:640], r0_t[:, 2048:2560], None),
            (w1_t[:, :],       r1_t[:, 0:512],     masks[2]),
            (w1_t[:, :],       r1_t[:, 512:1024],  masks[3]),
        ]
        for s, (lhs, rhs, m) in enumerate(slot_defs):
            ps = ppool.tile([128, 512], f32)
            nc.tensor.matmul(ps[:], lhs, rhs, start=True, stop=(m is None))
            if m is not None:
                nc.tensor.matmul(ps[:], id_t[:], m[:], start=False, stop=True)
            # y = min(max(x, 0), 1); then acc[:, s] = sum_n sqrt(y).
            # sum(relu(1 - sqrt(x))) = 512 - sum(sqrt(y)) since sqrt is
            # monotone; clamp keeps Sqrt in-range for any input.
            yt = spool.tile([128, 512], f32, tag="clamp")
            nc.vector.tensor_scalar(
                out=yt[:], in0=ps[:], scalar1=0.0, scalar2=1.0,
                op0=mybir.AluOpType.max, op1=mybir.AluOpType.min)
            sc = spool.tile([128, 512], f32, tag="sc")
            nc.scalar.activation(sc[:], yt[:],
                                 mybir.ActivationFunctionType.Sqrt,
                                 accum_out=acc_t[:, s:s + 1])
        nc.sync.dma_start(acc.ap(), acc_t[:])
    nc.compile()
    return nc


def _emulate_acc(in_map):
    """Numpy emulation of the device program (for sim/HW debugging)."""
    acc = np.zeros((128, NSLOT), np.float32)
    iota = np.arange(512, dtype=np.float32)[None, :]
    for s in range(NSLOT):
        if s < 5:
            lhs = in_map["w0"][:, 128 * s:128 * (s + 1)]
            rhs = in_map["r0"][:, 512 * s:512 * (s + 1)]
        else:
            lhs = in_map["w1"]
            rhs = in_map["r1"][:, 512 * (s - 5):512 * (s - 4)]
        x = lhs.T.astype(np.float32) @ rhs
        mcol = {0: 0, 1: 1, 5: 2, 6: 3}.get(s)
        if mcol is not None:
            th = in_map["thr"][:, mcol][:, None]
            x = x + (iota >= th).astype(np.float32) * BIG
        y = np.minimum(np.maximum(x, 0.0), 1.0)
        acc[:, s] = np.sqrt(y).sum(axis=1)
    return acc


def _reduce_loss(accs):
    """Host-side float64 reduction of per-core [128, NSLOT] accumulators."""
    S0 = 0.0
    S1 = 0.0
    for a in accs:
        a = np.asarray(a, np.float64)
        S0 += 5 * 128 * 512 - a[:, :5].sum()
        S1 += 2 * 128 * 512 - a[:, 5:].sum()
    loss = 0.5 * (2.0 * S0 / (N0 * (N0 - 1)) + 2.0 * S1 / (N1 * (N1 - 1)))
    return np.float32(loss)


def _get_program():
    global _cached_nc
    if _cached_nc is None:
        _cached_nc = _build_program()
    return _cached_nc


def kernel(kernels0, kernels1):
    global LAST_RESULT
    from concourse.bass_utils import run_bass_kernel_spmd

    W0, R0 = _features(kernels0, D0)
    W1, R1 = _features(kernels1, D1)
    in_maps = [_stage_core(c, W0, R0, W1, R1) for c in range(N_CORES)]
    nc = _get_program()
    res = run_bass_kernel_spmd(nc, in_maps, list(range(N_CORES)), trace=TRACE)
    LAST_RESULT = res
    accs = [res.results[c]["acc"] for c in range(N_CORES)]
    return _reduce_loss(accs)


# revision 6
# speedup vs baseline: 1.0639x; 1.0639x over previous
"""ContrastiveKernelLoss on 8 Trainium2 cores.

Math: for each layer, D[i,j] = ||I - inv(kn_i) @ kn_j||_F over the n x n pair
grid.  ||I - A B||_F^2 = d - 2*tr(A B) + ||A B||_F^2 expands into dot products
over 2*d^2+1 per-kernel features, so the whole grid is one skinny matmul:
  X = W^T R,  W rows = [-2*vec(inv_i); vec(inv_i^T inv_i); d],
              R rows = [vec(kn_j^T);  vec(kn_j kn_j^T);   1]
The loss only needs the strict lower triangle (combined = tril(D) + triu(D^T)),
so each core computes a balanced subset of lower-triangle 128x512 tiles
("slots").  Upper/diagonal positions are pushed out of the hinge by adding
BIG via a third matmul (identity lhsT @ mask), masks generated on-device from
iota >= per-row threshold.  Per tile: PE matmul -> ACT sqrt -> DVE min(s,1)
with fused row-sum.  sum(relu(1-D)) = n_cols - sum(min(s,1)) exactly, because
inactive entries contribute exact 1.0.  Host reduces in float64.
"""

import numpy as np
from contextlib import ExitStack

EPS = 1e-8
# Masked (upper/diagonal) grid entries get +BIG before sqrt.  Must dominate
# the f32 matmul rounding error of the diagonal cancellation, which can reach
# ~1e5 for near-singular kernels (feature products up to ~2e11).
BIG = 1e8
N0, D0 = 2048, 3
N1, D1 = 1024, 5
F0 = 2 * D0 * D0 + 1   # 19
F1 = 2 * D1 * D1 + 1   # 51
NSLOT = 7              # 5 layer0 slots + 2 layer1 slots per core
N_CORES = 8

TRACE = False          # test harness sets True to capture an NTFF profile
LAST_RESULT = None     # BassKernelResults of the most recent run

_cached_nc = None


def _features(kernels, d):
    """Per-kernel features (float64 prep, float32 out): W (F, n), R (F, n)."""
    k = np.asarray(kernels, np.float64)
    n = k.shape[0]
    norms = np.sqrt((k * k).sum(axis=(1, 2), keepdims=True)) + EPS
    kn = k / norms
    inv = np.linalg.inv(kn)
    G = np.einsum('iba,ibc->iac', inv, inv)        # inv^T @ inv
    H = np.einsum('iab,icb->iac', kn, kn)          # kn @ kn^T
    W = np.concatenate([
        -2.0 * inv.reshape(n, -1),
        G.reshape(n, -1),
        np.full((n, 1), float(d)),
    ], axis=1).T
    R = np.concatenate([
        np.transpose(kn, (0, 2, 1)).reshape(n, -1),
        H.reshape(n, -1),
        np.ones((n, 1)),
    ], axis=1).T
    return (np.ascontiguousarray(W, np.float32),
            np.ascontiguousarray(R, np.float32))


def _core_plan(core):
    """Slot list for one core: (layer, row_tile, col_chunk, thr_or_None).

    Layer0 (16 row tiles of 128, chunks of 512): core pairs (k,12+k)/(k,k+4)
    give every core 2 diagonal slots + 3 full slots.  Layer1: tile k diag slot
    + (full chunk | fully-masked dummy).  thr is the per-partition column
    threshold: columns n >= thr are excluded (+BIG).
    """
    tA, tB = (core, 12 + core) if core < 4 else (core, core + 4)
    slots = []
    for t in (tA, tB):                       # slots 0,1: layer0 diagonal tiles
        slots.append(("l0", t, t // 4, 128.0 * (t % 4) + np.arange(128)))
    fulls = [(tA, c) for c in range(tA // 4)] + [(tB, c) for c in range(tB // 4)]
    assert len(fulls) == 3
    for (t, c) in fulls:                     # slots 2-4: layer0 full chunks
        slots.append(("l0", t, c, None))
    t = core                                 # slot 5: layer1 diagonal tile
    slots.append(("l1", t, t // 4, 128.0 * (t % 4) + np.arange(128)))
    if core >= 4:                            # slot 6: real full chunk
        slots.append(("l1", t, 0, np.full(128, 600.0)))
    else:                                    # slot 6: fully-masked dummy
        slots.append(("l1", t, 0, np.zeros(128)))
    return slots


def _stage_core(core, W0, R0, W1, R1):
    """Build the per-core input map for the static SPMD program."""
    slots = _core_plan(core)
    w0 = np.empty((F0, 5 * 128), np.float32)
    r0 = np.empty((F0, 5 * 512), np.float32)
    thr = np.empty((128, 4), np.float32)
    mi = 0
    for s, (lay, t, c, th) in enumerate(slots[:5]):
        w0[:, 128 * s:128 * (s + 1)] = W0[:, 128 * t:128 * (t + 1)]
        r0[:, 512 * s:512 * (s + 1)] = R0[:, 512 * c:512 * (c + 1)]
        if th is not None:
            thr[:, mi] = th
            mi += 1
    assert mi == 2
    _, t5, c5, th5 = slots[5]
    _, t6, c6, th6 = slots[6]
    assert t6 == t5
    w1 = np.ascontiguousarray(W1[:, 128 * t5:128 * (t5 + 1)])
    r1 = np.empty((F1, 1024), np.float32)
    r1[:, 0:512] = R1[:, 512 * c5:512 * (c5 + 1)]
    r1[:, 512:1024] = R1[:, 512 * c6:512 * (c6 + 1)]
    thr[:, 2] = th5
    thr[:, 3] = th6
    return {
        "w0": w0, "r0": r0, "w1": w1, "r1": r1,
        "thr": thr, "ident": np.eye(128, dtype=np.float32),
    }


def _build_program():
    import concourse.bacc as bacc
    import concourse.tile as tile
    import concourse.mybir as mybir

    f32 = mybir.dt.float32
    nc = bacc.Bacc("TRN2")
    w0 = nc.declare_dram_parameter("w0", [F0, 5 * 128], f32, isOutput=False)
    r0 = nc.declare_dram_parameter("r0", [F0, 5 * 512], f32, isOutput=False)
    w1 = nc.declare_dram_parameter("w1", [F1, 128], f32, isOutput=False)
    r1 = nc.declare_dram_parameter("r1", [F1, 1024], f32, isOutput=False)
    thr = nc.declare_dram_parameter("thr", [128, 4], f32, isOutput=False)
    ident = nc.declare_dram_parameter("ident", [128, 128], f32, isOutput=False)
    acc = nc.declare_dram_parameter("acc", [128, NSLOT], f32, isOutput=True)

    with tile.TileContext(nc) as tc, ExitStack() as ctx:
        cpool = ctx.enter_context(tc.tile_pool(name="const", bufs=1))
        ppool = ctx.enter_context(
            tc.tile_pool(name="psum", bufs=4, space="PSUM"))
        spool = ctx.enter_context(tc.tile_pool(name="work", bufs=3))

        w0_t = cpool.tile([F0, 5 * 128], f32)
        nc.sync.dma_start(w0_t[:], w0.ap())
        r0_t = cpool.tile([F0, 5 * 512], f32)
        nc.sync.dma_start(r0_t[:], r0.ap())
        w1_t = cpool.tile([F1, 128], f32)
        nc.sync.dma_start(w1_t[:], w1.ap())
        r1_t = cpool.tile([F1, 1024], f32)
        nc.sync.dma_start(r1_t[:], r1.ap())
        thr_t = cpool.tile([128, 4], f32)
        nc.sync.dma_start(thr_t[:], thr.ap())
        id_t = cpool.tile([128, 128], f32)
        nc.sync.dma_start(id_t[:], ident.ap())

        iota_t = cpool.tile([128, 512], f32)
        nc.gpsimd.iota(iota_t[:], pattern=[[1, 512]], base=0,
                       channel_multiplier=0,
                       allow_small_or_imprecise_dtypes=True)
        masks = []
        for mi in range(4):
            m = cpool.tile([128, 512], f32, tag=f"mask{mi}")
            nc.vector.tensor_scalar(
                out=m[:], in0=iota_t[:], scalar1=thr_t[:, mi:mi + 1],
                scalar2=BIG, op0=mybir.AluOpType.is_ge,
                op1=mybir.AluOpType.mult)
            masks.append(m)

        acc_t = cpool.tile([128, NSLOT], f32)
        slot_defs = [
            (w0_t[:, 0:128],   r0_t[:, 0:512],     masks[0]),
            (w0_t[:, 128:256], r0_t[:, 512:1024],  masks[1]),
            (w0_t[:, 256:384], r0_t[:, 1024:1536], None),
            (w0_t[:, 384:512], r0_t[:, 1536:2048], None),
            (w0_t[:, 512:640], r0_t[:, 2048:2560], None),
            (w1_t[:, :],       r1_t[:, 0:512],     masks[2]),
            (w1_t[:, :],       r1_t[:, 512:1024],  masks[3]),
        ]
        for s, (lhs, rhs, m) in enumerate(slot_defs):
            ps = ppool.tile([128, 512], f32)
            nc.tensor.matmul(ps[:], lhs, rhs, start=True, stop=(m is None))
            if m is not None:
                nc.tensor.matmul(ps[:], id_t[:], m[:], start=False, stop=True)
            # acc[:, s] = sum_n min(sqrt(max(x, 0)), 1).  The min must come
            # AFTER the sqrt: the ACT sqrt table returns ~1.000002 at 1.0,
            # and the count-trick needs inactive entries to be exactly 1.
            yt = spool.tile([128, 512], f32, tag="clamp")
            nc.vector.tensor_scalar_max(out=yt[:], in0=ps[:], scalar1=0.0)
            sq = spool.tile([128, 512], f32, tag="sq")
            nc.scalar.activation(sq[:], yt[:],
                                 mybir.ActivationFunctionType.Sqrt)
            sc = spool.tile([128, 512], f32, tag="sc")
            nc.vector.tensor_scalar(
                out=sc[:], in0=sq[:], scalar1=1.0, scalar2=None,
                op0=mybir.AluOpType.min, op1=mybir.AluOpType.add,
                accum_out=acc_t[:, s:s + 1])
        nc.sync.dma_start(acc.ap(), acc_t[:])
    nc.compile()
    return nc


def _emulate_acc(in_map):
    """Numpy emulation of the device program (for sim/HW debugging)."""
    acc = np.zeros((128, NSLOT), np.float32)
    iota = np.arange(512, dtype=np.float32)[None, :]
    for s in range(NSLOT):
        if s < 5:
            lhs = in_map["w0"][:, 128 * s:128 * (s + 1)]
            rhs = in_map["r0"][:, 512 * s:512 * (s + 1)]
        else:
            lhs = in_map["w1"]
            rhs = in_map["r1"][:, 512 * (s - 5):512 * (s - 4)]
        x = lhs.T.astype(np.float32) @ rhs
        mcol = {0: 0, 1: 1, 5: 2, 6: 3}.get(s)
        if mcol is not None:
            th = in_map["thr"][:, mcol][:, None]
            x = x + (iota >= th).astype(np.float32) * BIG
        y = np.minimum(np.maximum(x, 0.0), 1.0)
        acc[:, s] = np.sqrt(y).sum(axis=1)
    return acc


def _reduce_loss(accs):
    """Host-side float64 reduction of per-core [128, NSLOT] accumulators."""
    S0 = 0.0
    S1 = 0.0
    for a in accs:
        a = np.asarray(a, np.float64)
        S0 += 5 * 128 * 512 - a[:, :5].sum()
        S1 += 2 * 128 * 512 - a[:, 5:].sum()
    loss = 0.5 * (2.0 * S0 / (N0 * (N0 - 1)) + 2.0 * S1 / (N1 * (N1 - 1)))
    return np.float32(loss)


def _get_program():
    global _cached_nc
    if _cached_nc is None:
        _cached_nc = _build_program()
    return _cached_nc


def kernel(kernels0, kernels1):
    global LAST_RESULT
    from concourse.bass_utils import run_bass_kernel_spmd

    W0, R0 = _features(kernels0, D0)
    W1, R1 = _features(kernels1, D1)
    in_maps = [_stage_core(c, W0, R0, W1, R1) for c in range(N_CORES)]
    nc = _get_program()
    res = run_bass_kernel_spmd(nc, in_maps, list(range(N_CORES)), trace=TRACE)
    LAST_RESULT = res
    accs = [res.results[c]["acc"] for c in range(N_CORES)]
    return _reduce_loss(accs)


# revision 7
# speedup vs baseline: 1.4690x; 1.3807x over previous
"""ContrastiveKernelLoss on 8 Trainium2 cores.

Math: for each layer, D[i,j] = ||I - inv(kn_i) @ kn_j||_F over the n x n pair
grid.  ||I - A B||_F^2 = d - 2*tr(A B) + ||A B||_F^2 expands into dot products
over per-kernel features, so the whole grid is one skinny matmul:
  X = W^T R,  W rows = [-2*vec(inv_i); vec(inv_i^T inv_i); d; 1],
              R rows = [vec(kn_j^T);  vec(kn_j kn_j^T);   1; m*_j]
The loss needs only the strict lower triangle (combined = tril(D)+triu(D^T)).
Each core computes a balanced subset of 128x512 lower-triangle tiles
("slots").  Masking of j >= i positions pushes X up by BIG so the hinge is 0:
  - column-only mask regions ride the extra rank-1 feature row (m*, staged
    per core — the program stays SPMD-uniform),
  - the 128x128 diagonal block is column-permuted to the tail of its chunk,
    where one shared strict-upper triangle constant is added via a second
    matmul (identity lhsT) — identical addresses on every core.
Per slot: PE matmul(s) -> ACT sqrt -> DVE min(s,1) with fused row-sum.
sum(relu(1-D)) = n_cols - sum(min(sqrt(X),1)) exactly, because inactive
entries contribute exact 1.0.  Host reduces in float64.
"""

import numpy as np
from contextlib import ExitStack

EPS = 1e-8
# Added to masked grid entries before sqrt.  Must dominate the f32 matmul
# rounding error of the diagonal cancellation, which can reach ~1e5 for
# near-singular kernels (feature products up to ~2e11).
BIG = 1e8
N0, D0 = 2048, 3
N1, D1 = 1024, 5
F0 = 2 * D0 * D0 + 2   # 20: features + bias row + mask row
F1 = 2 * D1 * D1 + 2   # 52
NSLOT = 7              # 5 layer0 slots + 2 layer1 slots per core
N_CORES = 8

TRACE = False          # test harness sets True to capture an NTFF profile
LAST_RESULT = None     # BassKernelResults of the most recent run

_cached_nc = None


def _features(kernels, d):
    """Per-kernel features (float64 prep, float32 out): W (F, n), R (F, n).

    Last row of R (the mask row) is zero here; _stage_core fills it per slot.
    """
    k = np.asarray(kernels, np.float64)
    n = k.shape[0]
    norms = np.sqrt((k * k).sum(axis=(1, 2), keepdims=True)) + EPS
    kn = k / norms
    inv = np.linalg.inv(kn)
    G = np.einsum('iba,ibc->iac', inv, inv)        # inv^T @ inv
    H = np.einsum('iab,icb->iac', kn, kn)          # kn @ kn^T
    W = np.concatenate([
        -2.0 * inv.reshape(n, -1),
        G.reshape(n, -1),
        np.full((n, 1), float(d)),
        np.ones((n, 1)),
    ], axis=1).T
    R = np.concatenate([
        np.transpose(kn, (0, 2, 1)).reshape(n, -1),
        H.reshape(n, -1),
        np.ones((n, 1)),
        np.zeros((n, 1)),
    ], axis=1).T
    return (np.ascontiguousarray(W, np.float32),
            np.ascontiguousarray(R, np.float32))


def _diag_slot(W, R, t):
    """Build (w, r) for a diagonal slot of row tile t: columns permuted to
    [left | right | block] with the mask row set to BIG on the right region.
    The shared triangle constant later masks n' >= m inside the block."""
    F = W.shape[0]
    c = t // 4
    L = 128 * (t % 4)
    r = np.empty((F, 512), np.float32)
    r[:, :L] = R[:, 512 * c:512 * c + L]                    # left: keep
    r[:, L:384] = R[:, 128 * t + 128:512 * (c + 1)]         # right: mask
    r[:, 384:] = R[:, 128 * t:128 * t + 128]                # diagonal block
    r[F - 1, :] = 0.0
    r[F - 1, L:384] = BIG
    return W[:, 128 * t:128 * (t + 1)], r


def _full_slot(W, R, t, c):
    return (W[:, 128 * t:128 * (t + 1)],
            np.ascontiguousarray(R[:, 512 * c:512 * (c + 1)]))


def _core_slots(core, W0, R0, W1, R1):
    """7 slots: list of (w [F,128], r [F,512], has_corr).  Slots 0,1 are
    layer0 diagonal tiles, 2-4 layer0 full chunks, 5 layer1 diagonal,
    6 layer1 full chunk (cores >= 4) or fully-masked dummy (cores < 4)."""
    tA, tB = (core, 12 + core) if core < 4 else (core, core + 4)
    slots = [(*_diag_slot(W0, R0, t), True) for t in (tA, tB)]
    fulls = [(tA, c) for c in range(tA // 4)] + [(tB, c) for c in range(tB // 4)]
    assert len(fulls) == 3
    slots += [(*_full_slot(W0, R0, t, c), False) for (t, c) in fulls]
    t = core
    slots.append((*_diag_slot(W1, R1, t), True))
    w6, r6 = _full_slot(W1, R1, t, 0)
    if core < 4:
        r6 = r6.copy()
        r6[F1 - 1, :] = BIG                     # dummy: fully masked
    slots.append((w6, r6, False))
    return slots


def _stage_core(core, W0, R0, W1, R1):
    """Per-core input map: one fused [F, 128+512] (w | r) block per slot."""
    slots = _core_slots(core, W0, R0, W1, R1)
    wr0 = np.empty((5, F0, 640), np.float32)
    wr1 = np.empty((2, F1, 640), np.float32)
    for s, (w, r, _) in enumerate(slots):
        dst = wr0[s] if s < 5 else wr1[s - 5]
        dst[:, :128] = w
        dst[:, 128:] = r
    return {"wr0": wr0, "wr1": wr1}


def _build_program():
    import concourse.bacc as bacc
    import concourse.tile as tile
    import concourse.mybir as mybir

    f32 = mybir.dt.float32
    nc = bacc.Bacc("TRN2")
    wr0 = nc.declare_dram_parameter("wr0", [5, F0, 640], f32, isOutput=False)
    wr1 = nc.declare_dram_parameter("wr1", [2, F1, 640], f32, isOutput=False)
    acc = nc.declare_dram_parameter("acc", [128, NSLOT], f32, isOutput=True)

    with tile.TileContext(nc) as tc, ExitStack() as ctx:
        cpool = ctx.enter_context(tc.tile_pool(name="const", bufs=1))
        ppool = ctx.enter_context(
            tc.tile_pool(name="psum", bufs=4, space="PSUM"))
        spool = ctx.enter_context(tc.tile_pool(name="work", bufs=3))

        # Per-slot fused (w | r) tiles; separate DMAs so slot s's matmul can
        # start as soon as its own slice lands (and queues parallelize).
        wr_tiles = []
        for s in range(NSLOT):
            F = F0 if s < 5 else F1
            src = wr0.ap()[s] if s < 5 else wr1.ap()[s - 5]
            t = cpool.tile([F, 640], f32, tag=f"wr{s}")
            nc.sync.dma_start(t[:], src)
            wr_tiles.append(t)

        # Shared constants, generated on-device: identity (corr lhsT) and
        # the strict-upper triangle TRI[k, n] = BIG * (n >= k).
        iota_c = cpool.tile([128, 128], f32)
        nc.gpsimd.iota(iota_c[:], pattern=[[1, 128]], base=0,
                       channel_multiplier=0,
                       allow_small_or_imprecise_dtypes=True)
        iota_p = cpool.tile([128, 1], f32)
        nc.gpsimd.iota(iota_p[:], pattern=[[1, 1]], base=0,
                       channel_multiplier=1,
                       allow_small_or_imprecise_dtypes=True)
        ident = cpool.tile([128, 128], f32)
        nc.vector.tensor_scalar(
            out=ident[:], in0=iota_c[:], scalar1=iota_p[:, 0:1], scalar2=None,
            op0=mybir.AluOpType.is_equal)
        tri = cpool.tile([128, 128], f32)
        nc.vector.tensor_scalar(
            out=tri[:], in0=iota_c[:], scalar1=iota_p[:, 0:1], scalar2=BIG,
            op0=mybir.AluOpType.is_ge, op1=mybir.AluOpType.mult)

        acc_t = cpool.tile([128, NSLOT], f32)
        has_corr = [True, True, False, False, False, True, False]
        for s in range(NSLOT):
            wr = wr_tiles[s]
            ps = ppool.tile([128, 512], f32)
            nc.tensor.matmul(ps[:], wr[:, 0:128], wr[:, 128:640],
                             start=True, stop=not has_corr[s])
            if has_corr[s]:
                nc.tensor.matmul(ps[:, 384:512], ident[:], tri[:],
                                 start=False, stop=True)
            sq = spool.tile([128, 512], f32, tag="sq")
            nc.scalar.activation(sq[:], ps[:],
                                 mybir.ActivationFunctionType.Sqrt)
            sc = spool.tile([128, 512], f32, tag="sc")
            nc.vector.tensor_scalar(
                out=sc[:], in0=sq[:], scalar1=1.0, scalar2=None,
                op0=mybir.AluOpType.min, op1=mybir.AluOpType.add,
                accum_out=acc_t[:, s:s + 1])
        nc.sync.dma_start(acc.ap(), acc_t[:])
    nc.compile()
    return nc


def _emulate_acc(in_map):
    """Numpy emulation of the device program (for sim/HW debugging)."""
    acc = np.zeros((128, NSLOT), np.float32)
    tri = (np.arange(128)[None, :] >= np.arange(128)[:, None]) * np.float32(BIG)
    has_corr = [True, True, False, False, False, True, False]
    for s in range(NSLOT):
        wr = in_map["wr0"][s] if s < 5 else in_map["wr1"][s - 5]
        x = wr[:, :128].T.astype(np.float32) @ wr[:, 128:]
        if has_corr[s]:
            x[:, 384:] += tri
        acc[:, s] = np.minimum(np.sqrt(x), 1.0).sum(axis=1)
    return acc


def _reduce_loss(accs):
    """Host-side float64 reduction of per-core [128, NSLOT] accumulators."""
    S0 = 0.0
    S1 = 0.0
    for a in accs:
        a = np.asarray(a, np.float64)
        S0 += 5 * 128 * 512 - a[:, :5].sum()
        S1 += 2 * 128 * 512 - a[:, 5:].sum()
    loss = 0.5 * (2.0 * S0 / (N0 * (N0 - 1)) + 2.0 * S1 / (N1 * (N1 - 1)))
    return np.float32(loss)


def _get_program():
    global _cached_nc
    if _cached_nc is None:
        _cached_nc = _build_program()
    return _cached_nc


def kernel(kernels0, kernels1):
    global LAST_RESULT
    from concourse.bass_utils import run_bass_kernel_spmd

    W0, R0 = _features(kernels0, D0)
    W1, R1 = _features(kernels1, D1)
    in_maps = [_stage_core(c, W0, R0, W1, R1) for c in range(N_CORES)]
    nc = _get_program()
    res = run_bass_kernel_spmd(nc, in_maps, list(range(N_CORES)), trace=TRACE)
    LAST_RESULT = res
    accs = [res.results[c]["acc"] for c in range(N_CORES)]
    return _reduce_loss(accs)


# revision 12
# speedup vs baseline: 2.0520x; 1.3969x over previous
"""ContrastiveKernelLoss on 8 Trainium2 cores.

Math: for each layer, D[i,j] = ||I - inv(kn_i) @ kn_j||_F over the n x n pair
grid.  ||I - A B||_F^2 = d - 2*tr(A B) + ||A B||_F^2 expands into dot products
over per-kernel features, so the whole grid is one skinny matmul:
  X = W^T R,  W rows = [-2*vec(inv_i); vec(inv_i^T inv_i); d; 1],
              R rows = [vec(kn_j^T);  vec(kn_j kn_j^T);   1; m*_j]
The loss needs only the strict lower triangle (combined = tril(D)+triu(D^T)).
Each core computes a balanced subset of 128x512 lower-triangle tiles
("slots").  Masking of j >= i positions pushes X up by BIG so the hinge is 0:
  - column-only mask regions ride the extra rank-1 feature row (m*, staged
    per core — the program stays SPMD-uniform),
  - the 128x128 diagonal block is column-permuted to the tail of its chunk,
    where one shared strict-upper triangle constant is added via a second
    matmul (identity lhsT) — identical addresses on every core.
Per slot: PE matmul(s) -> ACT sqrt -> DVE min(s,1) with fused row-sum.
sum(relu(1-D)) = n_cols - sum(min(sqrt(X),1)) exactly, because inactive
entries contribute exact 1.0.  Host reduces in float64.
"""

import numpy as np
from contextlib import ExitStack

EPS = 1e-8
# Added to masked grid entries before sqrt.  Must dominate the f32 matmul
# rounding error of the diagonal cancellation, which can reach ~1e5 for
# near-singular kernels (feature products up to ~2e11).
BIG = 1e8
N0, D0 = 2048, 3
N1, D1 = 1024, 5
F0 = 2 * D0 * D0 + 2   # 20: features + bias row + mask row
F1 = 2 * D1 * D1 + 2   # 52
NSLOT = 7              # 5 layer0 slots + 2 layer1 slots per core
N_CORES = 8

TRACE = False          # test harness sets True to capture an NTFF profile
LAST_RESULT = None     # BassKernelResults of the most recent run

_cached_nc = None


def _features(kernels, d):
    """Per-kernel features (float64 prep, float32 out): W (F, n), R (F, n).

    Last row of R (the mask row) is zero here; _stage_core fills it per slot.
    """
    k = np.asarray(kernels, np.float64)
    n = k.shape[0]
    norms = np.sqrt((k * k).sum(axis=(1, 2), keepdims=True)) + EPS
    kn = k / norms
    inv = np.linalg.inv(kn)
    G = np.einsum('iba,ibc->iac', inv, inv)        # inv^T @ inv
    H = np.einsum('iab,icb->iac', kn, kn)          # kn @ kn^T
    W = np.concatenate([
        -2.0 * inv.reshape(n, -1),
        G.reshape(n, -1),
        np.full((n, 1), float(d)),
        np.ones((n, 1)),
    ], axis=1).T
    R = np.concatenate([
        np.transpose(kn, (0, 2, 1)).reshape(n, -1),
        H.reshape(n, -1),
        np.ones((n, 1)),
        np.zeros((n, 1)),
    ], axis=1).T
    return (np.ascontiguousarray(W, np.float32),
            np.ascontiguousarray(R, np.float32))


def _diag_slot(W, R, t):
    """Build (w, r) for a diagonal slot of row tile t: columns permuted to
    [left | right | block] with the mask row set to BIG on the right region.
    The shared triangle constant later masks n' >= m inside the block."""
    F = W.shape[0]
    c = t // 4
    L = 128 * (t % 4)
    r = np.empty((F, 512), np.float32)
    r[:, :L] = R[:, 512 * c:512 * c + L]                    # left: keep
    r[:, L:384] = R[:, 128 * t + 128:512 * (c + 1)]         # right: mask
    r[:, 384:] = R[:, 128 * t:128 * t + 128]                # diagonal block
    r[F - 1, :] = 0.0
    r[F - 1, L:384] = BIG
    return W[:, 128 * t:128 * (t + 1)], r


def _full_slot(W, R, t, c):
    return (W[:, 128 * t:128 * (t + 1)],
            np.ascontiguousarray(R[:, 512 * c:512 * (c + 1)]))


def _core_slots(core, W0, R0, W1, R1):
    """7 slots: list of (w [F,128], r [F,512], has_corr).  Slots 0,1 are
    layer0 diagonal tiles, 2-4 layer0 full chunks, 5 layer1 diagonal,
    6 layer1 full chunk (cores >= 4) or fully-masked dummy (cores < 4)."""
    tA, tB = (core, 12 + core) if core < 4 else (core, core + 4)
    slots = [(*_diag_slot(W0, R0, t), True) for t in (tA, tB)]
    fulls = [(tA, c) for c in range(tA // 4)] + [(tB, c) for c in range(tB // 4)]
    assert len(fulls) == 3
    slots += [(*_full_slot(W0, R0, t, c), False) for (t, c) in fulls]
    t = core
    slots.append((*_diag_slot(W1, R1, t), True))
    w6, r6 = _full_slot(W1, R1, t, 0)
    if core < 4:
        r6 = r6.copy()
        r6[F1 - 1, :] = BIG                     # dummy: fully masked
    slots.append((w6, r6, False))
    return slots


def _stage_core(core, W0, R0, W1, R1):
    """Per-core input map: one fused [F, 128+512] (w | r) block per slot."""
    slots = _core_slots(core, W0, R0, W1, R1)
    wr0 = np.empty((5, F0, 640), np.float32)
    wr1 = np.empty((2, F1, 640), np.float32)
    for s, (w, r, _) in enumerate(slots):
        dst = wr0[s] if s < 5 else wr1[s - 5]
        dst[:, :128] = w
        dst[:, 128:] = r
    return {"wr0": wr0, "wr1": wr1}


def _build_program():
    import concourse.bacc as bacc
    import concourse.tile as tile
    import concourse.mybir as mybir

    f32 = mybir.dt.float32
    f32r = mybir.dt.float32r
    nc = bacc.Bacc("TRN2")
    wr0 = nc.declare_dram_parameter("wr0", [5, F0, 640], f32r, isOutput=False)
    wr1 = nc.declare_dram_parameter("wr1", [2, F1, 640], f32r, isOutput=False)
    acc = nc.declare_dram_parameter("acc", [NSLOT, 1], f32, isOutput=True)

    with tile.TileContext(nc) as tc, ExitStack() as ctx:
        cpool = ctx.enter_context(tc.tile_pool(name="const", bufs=1))
        ppool = ctx.enter_context(
            tc.tile_pool(name="psum", bufs=4, space="PSUM"))
        spool = ctx.enter_context(tc.tile_pool(name="work", bufs=3))

        # Per-slot fused (w | r) tiles; separate DMAs so slot s's matmul can
        # start as soon as its own slice lands (and queues parallelize).
        wr_tiles = []
        for s in range(NSLOT):
            F = F0 if s < 5 else F1
            src = wr0.ap()[s] if s < 5 else wr1.ap()[s - 5]
            t = cpool.tile([F, 640], f32r, tag=f"wr{s}")
            nc.sync.dma_start(t[:], src)
            wr_tiles.append(t)

        # Shared constants, generated on-device: identity (corr lhsT) and
        # the strict-upper triangle TRI[k, n] = BIG * (n >= k).
        iota_c = cpool.tile([128, 128], f32)
        nc.gpsimd.iota(iota_c[:], pattern=[[1, 128]], base=0,
                       channel_multiplier=0,
                       allow_small_or_imprecise_dtypes=True)
        iota_p = cpool.tile([128, 1], f32)
        nc.gpsimd.iota(iota_p[:], pattern=[[1, 1]], base=0,
                       channel_multiplier=1,
                       allow_small_or_imprecise_dtypes=True)
        ident = cpool.tile([128, 128], f32r)
        nc.vector.tensor_scalar(
            out=ident[:], in0=iota_c[:], scalar1=iota_p[:, 0:1], scalar2=None,
            op0=mybir.AluOpType.is_equal)
        tri = cpool.tile([128, 128], f32r)
        nc.vector.tensor_scalar(
            out=tri[:], in0=iota_c[:], scalar1=iota_p[:, 0:1], scalar2=BIG,
            op0=mybir.AluOpType.is_ge, op1=mybir.AluOpType.mult)

        ones = cpool.tile([128, 1], f32)
        nc.gpsimd.memset(ones[:], 1.0)

        acc_t = cpool.tile([128, NSLOT], f32)
        has_corr = [True, True, False, False, False, True, False]
        for s in range(NSLOT):
            wr = wr_tiles[s]
            ps = ppool.tile([128, 512], f32)
            nc.tensor.matmul(ps[:], wr[:, 0:128], wr[:, 128:640],
                             start=True, stop=not has_corr[s])
            if has_corr[s]:
                nc.tensor.matmul(ps[:, 384:512], ident[:], tri[:],
                                 start=False, stop=True)
            sq = spool.tile([128, 512], f32, tag="sq")
            nc.scalar.activation(sq[:], ps[:],
                                 mybir.ActivationFunctionType.Sqrt)
            sc = spool.tile([128, 512], f32, tag="sc")
            nc.vector.tensor_scalar(
                out=sc[:], in0=sq[:], scalar1=1.0, scalar2=None,
                op0=mybir.AluOpType.min, op1=mybir.AluOpType.add,
                accum_out=acc_t[:, s:s + 1])
        # Partition-reduce acc on PE (out[s] = sum_p acc[p, s]) so the output
        # DMA is NSLOT packets instead of 128.
        # Full-fp32 matmul here: fp32r's ~10-bit mantissa would cost ~ulp(64)
        # on these ~65536-magnitude sums.
        acc_ps = ppool.tile([NSLOT, 1], f32, tag="accps")
        nc.tensor.matmul(acc_ps[:], acc_t[:], ones[:], start=True, stop=True)
        acc_sb = cpool.tile([NSLOT, 1], f32)
        nc.scalar.copy(acc_sb[:], acc_ps[:])
        nc.sync.dma_start(acc.ap(), acc_sb[:])
    nc.compile()
    return nc


def _emulate_acc(in_map):
    """Numpy emulation of the device program (for sim/HW debugging)."""
    acc = np.zeros((128, NSLOT), np.float32)
    tri = (np.arange(128)[None, :] >= np.arange(128)[:, None]) * np.float32(BIG)
    has_corr = [True, True, False, False, False, True, False]
    for s in range(NSLOT):
        wr = in_map["wr0"][s] if s < 5 else in_map["wr1"][s - 5]
        x = wr[:, :128].T.astype(np.float32) @ wr[:, 128:]
        if has_corr[s]:
            x[:, 384:] += tri
        acc[:, s] = np.minimum(np.sqrt(x), 1.0).sum(axis=1)
    return acc.sum(axis=0).reshape(NSLOT, 1)


def _reduce_loss(accs):
    """Host-side float64 reduction of per-core [NSLOT, 1] accumulators."""
    S0 = 0.0
    S1 = 0.0
    for a in accs:
        a = np.asarray(a, np.float64).reshape(NSLOT)
        S0 += 5 * 128 * 512 - a[:5].sum()
        S1 += 2 * 128 * 512 - a[5:].sum()
    loss = 0.5 * (2.0 * S0 / (N0 * (N0 - 1)) + 2.0 * S1 / (N1 * (N1 - 1)))
    return np.float32(loss)


def _get_program():
    global _cached_nc
    if _cached_nc is None:
        _cached_nc = _build_program()
    return _cached_nc


def kernel(kernels0, kernels1):
    global LAST_RESULT
    from concourse.bass_utils import run_bass_kernel_spmd

    W0, R0 = _features(kernels0, D0)
    W1, R1 = _features(kernels1, D1)
    in_maps = [_stage_core(c, W0, R0, W1, R1) for c in range(N_CORES)]
    nc = _get_program()
    res = run_bass_kernel_spmd(nc, in_maps, list(range(N_CORES)), trace=TRACE)
    LAST_RESULT = res
    accs = [res.results[c]["acc"] for c in range(N_CORES)]
    return _reduce_loss(accs)
